# revision 1
# baseline (speedup 1.0000x reference)
"""Trainium2 Bass kernel for Attention3D (B=2, N=1024, C=768, H=12, HID=64).

Sharding: 8 cores = 2 batches x 4 query-slices of 256 rows.
Each core computes, for its (batch, i-slice):
  - k^T, v for all 1024 tokens (x-stationary / w-stationary matmuls)
  - continuous 3D rel-pos bias for its (256 i x 1024 j) pairs, all 12 heads:
      hidden^T via DVE fused add+relu (2 query rows packed into 128 partitions),
      w2 contraction on PE (2-pair-packed, M=24), reshape via DRAM roundtrip
  - scores = q k^T + bias, softmax over free axis (max via DVE, exp+sum via ACT)
  - PE transpose of normalized exp-scores, then AV matmul and output projection
Host: input marshalling (transposes/scaling/packing) + concat + proj_b add.
"""

import os
import sys

for _p in ("/opt/trn_rl_repo",):
    if _p not in sys.path:
        sys.path.insert(0, _p)

import numpy as np
import ml_dtypes

from contextlib import ExitStack

import concourse.bass as bass
import concourse.bacc as bacc
import concourse.mybir as mybir
import concourse.tile as tile
from concourse import bass_utils
from concourse.masks import make_identity

BF16 = mybir.dt.bfloat16
F32 = mybir.dt.float32
AX = mybir.AxisListType
ALU = mybir.AluOpType
ACTF = mybir.ActivationFunctionType

B, N, C, H, HID = 2, 1024, 768, 12, 64
HD = C // H  # 64
NSLICE = 4          # query slices per batch
I_LEN = N // NSLICE  # 256
P = 128

LAST_EXEC_NS = None
LAST_RESULTS = None

_CACHE = {}


def _build_program():
    nc = bacc.Bacc(
        "TRN2",
        target_bir_lowering=False,
        debug=False,
        enable_asserts=False,
        num_devices=8,
    )

    # DRAM I/O (per-core inputs; same names for all cores)
    xT = nc.dram_tensor("xT", [C, N], BF16, kind="ExternalInput").ap()
    xTq = nc.dram_tensor("xTq", [C, I_LEN], BF16, kind="ExternalInput").ap()
    qwT = nc.dram_tensor("qwT", [C, C], BF16, kind="ExternalInput").ap()
    kwT = nc.dram_tensor("kwT", [C, C], BF16, kind="ExternalInput").ap()
    vwT = nc.dram_tensor("vwT", [C, C], BF16, kind="ExternalInput").ap()
    pwT = nc.dram_tensor("pwT", [C, C], BF16, kind="ExternalInput").ap()
    ptn2 = nc.dram_tensor("ptn2", [P, N], BF16, kind="ExternalInput").ap()
    at2 = nc.dram_tensor("at2", [P, I_LEN // 2], F32, kind="ExternalInput").ap()
    w2pk = nc.dram_tensor("w2pk", [P, 2 * H], BF16, kind="ExternalInput").ap()
    out = nc.dram_tensor("out", [I_LEN, C], F32, kind="ExternalOutput").ap()

    with tile.TileContext(nc) as tc, ExitStack() as ctx:
        consts = ctx.enter_context(tc.tile_pool(name="consts", bufs=1))
        dram = ctx.enter_context(tc.tile_pool(name="dram", bufs=1, space="DRAM"))
        psq = ctx.enter_context(tc.tile_pool(name="psq", bufs=2, space="PSUM"))
        psw2 = ctx.enter_context(tc.tile_pool(name="psw2", bufs=3, space="PSUM"))
        pss = psq
        pstr = ctx.enter_context(tc.tile_pool(name="pstr", bufs=2, space="PSUM"))
        psav = ctx.enter_context(tc.tile_pool(name="psav", bufs=1, space="PSUM"))
        hidp = ctx.enter_context(tc.tile_pool(name="hidp", bufs=4))
        stg = ctx.enter_context(tc.tile_pool(name="stg", bufs=2))
        biasp = ctx.enter_context(tc.tile_pool(name="biasp", bufs=3))
        sbp = ctx.enter_context(tc.tile_pool(name="sbp", bufs=3))
        esp = ctx.enter_context(tc.tile_pool(name="esp", bufs=3))
        etp = ctx.enter_context(tc.tile_pool(name="etp", bufs=3))
        smallp = ctx.enter_context(tc.tile_pool(name="smallp", bufs=8))
        outp = ctx.enter_context(tc.tile_pool(name="outp", bufs=2))

        # ---- constants / staged inputs in SBUF ----
        xT_sb = consts.tile([P, 6, N], BF16)
        nc.sync.dma_start(xT_sb[:], xT.rearrange("(c p) n -> p c n", p=P))
        xTq_sb = consts.tile([P, 6, I_LEN], BF16)
        nc.sync.dma_start(xTq_sb[:], xTq.rearrange("(c p) n -> p c n", p=P))
        qwT_sb = consts.tile([P, 6, C], BF16)
        nc.sync.dma_start(qwT_sb[:], qwT.rearrange("(c p) f -> p c f", p=P))
        kwT_sb = consts.tile([P, 6, C], BF16)
        nc.sync.dma_start(kwT_sb[:], kwT.rearrange("(c p) f -> p c f", p=P))
        vwT_sb = consts.tile([P, 6, C], BF16)
        nc.sync.dma_start(vwT_sb[:], vwT.rearrange("(c p) f -> p c f", p=P))
        pwT_sb = consts.tile([P, 6, C], BF16)
        nc.sync.dma_start(pwT_sb[:], pwT.rearrange("(c p) f -> p c f", p=P))
        ptn2_sb = consts.tile([P, N], BF16)
        nc.sync.dma_start(ptn2_sb[:], ptn2)
        at2_sb = consts.tile([P, I_LEN // 2], F32)
        nc.sync.dma_start(at2_sb[:], at2)
        w2pk_sb = consts.tile([P, 2 * H], BF16)
        nc.sync.dma_start(w2pk_sb[:], w2pk)
        ident = consts.tile([P, P], BF16)
        make_identity(nc, ident[:])

        kT_sb = consts.tile([P, 6, N], BF16)
        v_sb = consts.tile([P, 8, C], BF16)
        qT_sb = consts.tile([P, 6, I_LEN], BF16)
        attnT_sb = consts.tile([P, 6, I_LEN], BF16)

        bias_dram = dram.tile([I_LEN, H, N], BF16)

        # ---- phase 2: rel-pos bias ----
        # hidden^T for 2 packed query rows: relu(AT2[:, ip] + ptn2)
        G = 8  # ip-group size per staging buffer / store DMA
        bias_view = bias_dram[:].rearrange("(g two) h j -> two g h j", two=2)
        for g0 in range(0, I_LEN // 2, G):
            sg = stg.tile([2 * H, G, N], BF16, tag="stg")
            for gi in range(G):
                ip = g0 + gi
                h2 = hidp.tile([P, N], BF16, tag="h2")
                nc.vector.tensor_scalar(
                    h2[:], ptn2_sb[:], at2_sb[:, ip:ip + 1], 0.0,
                    ALU.add, ALU.max,
                )
                for jh in range(2):
                    ps = psw2.tile([2 * H, 512], F32, tag="w2")
                    nc.tensor.matmul(
                        ps[:], w2pk_sb[:], h2[:, jh * 512:(jh + 1) * 512],
                        start=True, stop=True,
                    )
                    if (gi + jh) % 2 == 0:
                        nc.vector.tensor_copy(sg[:, gi, jh * 512:(jh + 1) * 512], ps[:])
                    else:
                        nc.scalar.copy(sg[:, gi, jh * 512:(jh + 1) * 512], ps[:])
            for a in range(2):
                nc.gpsimd.dma_start(
                    bias_view[a, g0:g0 + G, :, :].rearrange("g h j -> h g j"),
                    sg[a * H:(a + 1) * H, :, :].rearrange("h g j -> h g j"),
                )

        # ---- phase 1: qkv projections ----
        # k^T [768 feat, 1024 tok]: lhsT = kwT chunk, rhs = xT chunk
        for fc in range(6):
            for jh in range(2):
                ps = psq.tile([P, 512], F32, tag="psq")
                for cc in range(6):
                    nc.tensor.matmul(
                        ps[:],
                        kwT_sb[:, cc, fc * P:(fc + 1) * P],
                        xT_sb[:, cc, jh * 512:(jh + 1) * 512],
                        start=(cc == 0),
                        stop=(cc == 5),
                    )
                nc.scalar.copy(kT_sb[:, fc, jh * 512:(jh + 1) * 512], ps[:])
        # v natural [1024 tok, 768 feat]: lhsT = xT chunk (tokens as M), rhs = vwT
        for tci in range(8):
            for oh in range(2):
                ps = psq.tile([P, 384], F32, tag="psq")
                for cc in range(6):
                    nc.tensor.matmul(
                        ps[:],
                        xT_sb[:, cc, tci * P:(tci + 1) * P],
                        vwT_sb[:, cc, oh * 384:(oh + 1) * 384],
                        start=(cc == 0),
                        stop=(cc == 5),
                    )
                nc.scalar.copy(v_sb[:, tci, oh * 384:(oh + 1) * 384], ps[:])
        # q^T for the core's i-slice [768 feat, 256]
        for fc in range(6):
            ps = psq.tile([P, I_LEN], F32, tag="psq")
            for cc in range(6):
                nc.tensor.matmul(
                    ps[:],
                    qwT_sb[:, cc, fc * P:(fc + 1) * P],
                    xTq_sb[:, cc, :],
                    start=(cc == 0),
                    stop=(cc == 5),
                )
            nc.scalar.copy(qT_sb[:, fc, :], ps[:])

        # ---- phase 3: attention per head ----
        for h in range(12):
            off = (h % 2) * 64
            fc = h // 2
            esn = []
            for ic in range(2):
                Sb = sbp.tile([P, N], F32, tag="Sb")
                for jh in range(2):
                    bt = biasp.tile([P, 512], BF16, tag="bias")
                    nc.sync.dma_start(
                        bt[:],
                        bias_dram[ic * P:(ic + 1) * P, h, jh * 512:(jh + 1) * 512],
                    )
                    ps = pss.tile([P, 512], F32, tag="psq")
                    nc.tensor.matmul(
                        ps[:],
                        qT_sb[off:off + 64, fc, ic * P:(ic + 1) * P],
                        kT_sb[off:off + 64, fc, jh * 512:(jh + 1) * 512],
                        start=True, stop=True,
                    )
                    nc.vector.tensor_add(Sb[:, jh * 512:(jh + 1) * 512], ps[:], bt[:])
                es = esp.tile([P, N], BF16, tag="es")
                sm = smallp.tile([P, 1], F32, tag="sm")
                nc.scalar.activation(
                    es[:], Sb[:], ACTF.Exp, bias=0.0, scale=1.0, accum_out=sm[:],
                )
                rc = smallp.tile([P, 1], F32, tag="rc")
                nc.vector.reciprocal(rc[:], sm[:])
                en = esp.tile([P, N], BF16, tag="en")
                nc.vector.tensor_scalar_mul(en[:], es[:], rc[:])
                esn.append(en)
            # AV with PE-transposed probabilities
            pso = psav.tile([64, I_LEN], F32, tag="av")
            for jc in range(8):
                eT = etp.tile([P, I_LEN], BF16, tag="eT")
                for ic in range(2):
                    pst = pstr.tile([P, P], BF16, tag="tr")
                    nc.tensor.transpose(
                        pst[:], esn[ic][:, jc * P:(jc + 1) * P], ident[:]
                    )
                    if ic == 0:
                        nc.vector.tensor_copy(eT[:, 0:P], pst[:])
                    else:
                        nc.scalar.copy(eT[:, P:2 * P], pst[:])
                nc.tensor.matmul(
                    pso[:],
                    v_sb[:, jc, h * 64:(h + 1) * 64],
                    eT[:],
                    start=(jc == 0),
                    stop=(jc == 7),
                )
            nc.vector.tensor_copy(attnT_sb[off:off + 64, fc, :], pso[:])

        # ---- phase 4: output projection ----
        for ic in range(2):
            for oh in range(2):
                ps = psq.tile([P, 384], F32, tag="psq")
                for cc in range(6):
                    nc.tensor.matmul(
                        ps[:],
                        attnT_sb[:, cc, ic * P:(ic + 1) * P],
                        pwT_sb[:, cc, oh * 384:(oh + 1) * 384],
                        start=(cc == 0),
                        stop=(cc == 5),
                    )
                ot = outp.tile([P, 384], F32, tag="ot")
                nc.scalar.copy(ot[:], ps[:])
                nc.gpsimd.dma_start(
                    out[ic * P:(ic + 1) * P, oh * 384:(oh + 1) * 384], ot[:]
                )

    nc.compile()
    return nc


def _prep_inputs(x, coords_3d, qkv_w, proj_w, mlp_w1, mlp_b1, mlp_w2):
    bf = ml_dtypes.bfloat16
    in_maps = []
    qw = (qkv_w[0:C] * (HD ** -0.5)).astype(np.float32)
    kw = qkv_w[C:2 * C]
    vw = qkv_w[2 * C:3 * C]
    qwT = np.ascontiguousarray(qw.T).astype(bf)
    kwT = np.ascontiguousarray(kw.T).astype(bf)
    vwT = np.ascontiguousarray(vw.T).astype(bf)
    pwT = np.ascontiguousarray(proj_w.T).astype(bf)
    w2pk = np.zeros((P, 2 * H), np.float32)
    w2pk[0:HID, 0:H] = mlp_w2.T
    w2pk[HID:2 * HID, H:2 * H] = mlp_w2.T
    w2pk = w2pk.astype(bf)

    for b in range(B):
        cb = coords_3d[b].astype(np.float32)
        mv = cb.max(axis=0) - cb.min(axis=0) + 1e-6
        cn = cb / mv
        Pm = cn @ mlp_w1.T.astype(np.float32)          # (1024, 64)
        Am = Pm + mlp_b1.astype(np.float32)            # (1024, 64)
        ptn2 = np.empty((P, N), np.float32)
        ptn2[0:HID] = -Pm.T
        ptn2[HID:2 * HID] = -Pm.T
        ptn2 = ptn2.astype(bf)
        xT_b = np.ascontiguousarray(x[b].T).astype(bf)  # (768, 1024)
        for s in range(NSLICE):
            i0 = s * I_LEN
            at2 = np.empty((P, I_LEN // 2), np.float32)
            Al = Am[i0:i0 + I_LEN]
            at2[0:HID] = Al[0::2].T
            at2[HID:2 * HID] = Al[1::2].T
            xTq = np.ascontiguousarray(x[b, i0:i0 + I_LEN].T).astype(bf)
            in_maps.append({
                "xT": xT_b,
                "xTq": xTq,
                "qwT": qwT,
                "kwT": kwT,
                "vwT": vwT,
                "pwT": pwT,
                "ptn2": ptn2,
                "at2": at2.astype(np.float32),
                "w2pk": w2pk,
            })
    return in_maps


def kernel(x, coords_3d, qkv_w, proj_w, proj_b, mlp_w1, mlp_b1, mlp_w2, mlp_b2):
    global LAST_EXEC_NS, LAST_RESULTS
    x = np.asarray(x, np.float32)
    coords_3d = np.asarray(coords_3d, np.float32)
    qkv_w = np.asarray(qkv_w, np.float32)
    proj_w = np.asarray(proj_w, np.float32)
    proj_b = np.asarray(proj_b, np.float32)
    mlp_w1 = np.asarray(mlp_w1, np.float32)
    mlp_b1 = np.asarray(mlp_b1, np.float32)
    mlp_w2 = np.asarray(mlp_w2, np.float32)

    if "nc" not in _CACHE:
        _CACHE["nc"] = _build_program()
    nc = _CACHE["nc"]

    in_maps = _prep_inputs(x, coords_3d, qkv_w, proj_w, mlp_w1, mlp_b1, mlp_w2)
    trace = bool(int(os.environ.get("KERNEL_TRACE", "0")))
    res = bass_utils.run_bass_kernel_spmd(
        nc, in_maps, list(range(8)), trace=trace
    )
    LAST_EXEC_NS = res.exec_time_ns
    LAST_RESULTS = res
    full = np.empty((B, N, C), np.float32)
    ci = 0
    for b in range(B):
        for s in range(NSLICE):
            full[b, s * I_LEN:(s + 1) * I_LEN] = res.results[ci]["out"]
            ci += 1
    full += proj_b[None, None, :]
    return full



# revision 4
# speedup vs baseline: 2.5172x; 2.5172x over previous
"""Trainium2 Bass kernel for Attention3D (B=2, N=1024, C=768, H=12, HID=64).

Sharding: 8 cores = 2 batches x 4 query-slices of 256 rows.

Design (transposed attention, bias fused into scores via PSUM accumulation):
  - x is token-rotated per core so its i-slice is at columns 0:256 (no xTq).
  - kT [feat, tok], v (natural, with a ones-column per head for softmax
    denominators) and qT computed on PE.
  - hidden h2[(par,d), j] = relu(A[i,d] - P[d,j]) per i-pair via DVE
    tensor_scalar (add+max, 4x mode).
  - scores^T [j, i] per head and rel-pos bias accumulate into the SAME psum
    bank: bias matmuls use h2 as stationary weights (M=128 tokens j) and a
    packed w2 [128, 24] as the 24-column moving operand; scores matmuls
    (lhsT = kT chunk, rhs = qT chunk) then accumulate on top.  Layout within
    a bank: col = il*12 + h for 32 i's x 12 heads.
  - exp on ACT (no max subtraction; logits are O(1)) -> expST staging bf16.
  - AV: lhsT = expST [j, i-chunk], rhs = [v_h | 1] -> O[i, d] + row sums in
    psum; normalize by reciprocal sums during psum->sbuf copy.
  - out projection from PE-transposed O, direct accumulation over 6 c-chunks.
Host: input marshalling (transpose/rotate/scale/pack) + concat + proj_b add.
"""

import os
import sys

for _p in ("/opt/trn_rl_repo",):
    if _p not in sys.path:
        sys.path.insert(0, _p)

import numpy as np
import ml_dtypes

from contextlib import ExitStack

import concourse.bass as bass
import concourse.bacc as bacc
import concourse.mybir as mybir
import concourse.tile as tile
from concourse import bass_utils
from concourse.masks import make_identity

BF16 = mybir.dt.bfloat16
F32 = mybir.dt.float32
ALU = mybir.AluOpType
ACTF = mybir.ActivationFunctionType

B, N, C, H, HID = 2, 1024, 768, 12, 64
HD = C // H  # 64
NSLICE = 4
I_LEN = N // NSLICE  # 256
P = 128

LAST_EXEC_NS = None
LAST_RESULTS = None

_CACHE = {}


def _build_program():
    nc = bacc.Bacc(
        "TRN2",
        target_bir_lowering=False,
        debug=False,
        enable_asserts=False,
        num_devices=8,
    )

    # DRAM I/O (per-core inputs; same names for all cores)
    xT = nc.dram_tensor("xT", [C, N], BF16, kind="ExternalInput").ap()
    qwT = nc.dram_tensor("qwT", [C, C], BF16, kind="ExternalInput").ap()
    kwT = nc.dram_tensor("kwT", [C, C], BF16, kind="ExternalInput").ap()
    vwT = nc.dram_tensor("vwT", [C, C], BF16, kind="ExternalInput").ap()
    pwT = nc.dram_tensor("pwT", [C, C], BF16, kind="ExternalInput").ap()
    ptn2 = nc.dram_tensor("ptn2", [P, N], BF16, kind="ExternalInput").ap()
    at2 = nc.dram_tensor("at2", [P, I_LEN // 2], F32, kind="ExternalInput").ap()
    w2pk = nc.dram_tensor("w2pk", [P, 2 * H], BF16, kind="ExternalInput").ap()
    out = nc.dram_tensor("out", [I_LEN, C], F32, kind="ExternalOutput").ap()

    with tile.TileContext(nc) as tc, ExitStack() as ctx:
        consts = ctx.enter_context(tc.tile_pool(name="consts", bufs=1))
        h2p = ctx.enter_context(tc.tile_pool(name="h2p", bufs=6))
        expp = ctx.enter_context(tc.tile_pool(name="expp", bufs=2))
        anp = ctx.enter_context(tc.tile_pool(name="anp", bufs=2))
        outp = ctx.enter_context(tc.tile_pool(name="outp", bufs=2))
        rcp = ctx.enter_context(tc.tile_pool(name="rcp", bufs=2))
        ssp = ctx.enter_context(tc.tile_pool(name="ssp", bufs=2, space="PSUM"))
        avp = ctx.enter_context(tc.tile_pool(name="avp", bufs=1, space="PSUM"))
        mmp = ctx.enter_context(tc.tile_pool(name="mmp", bufs=2, space="PSUM"))

        # ---- staged inputs in SBUF ----
        xT_sb = consts.tile([P, 6, N], BF16)
        for cc in range(6):
            for jh in range(2):
                nc.sync.dma_start(
                    xT_sb[:, cc, jh * 512:(jh + 1) * 512],
                    xT.rearrange("(c p) n -> p c n", p=P)[
                        :, cc, jh * 512:(jh + 1) * 512
                    ],
                )
        qwT_sb = consts.tile([P, 6, C], BF16)
        kwT_sb = consts.tile([P, 6, C], BF16)
        vwT_sb = consts.tile([P, 6, C], BF16)
        pwT_sb = consts.tile([P, 6, C], BF16)
        for w_sb, w_dram in ((kwT_sb, kwT), (qwT_sb, qwT), (vwT_sb, vwT),
                             (pwT_sb, pwT)):
            for cc in range(6):
                nc.sync.dma_start(
                    w_sb[:, cc, :],
                    w_dram.rearrange("(c p) f -> p c f", p=P)[:, cc, :],
                )
        ptn2_sb = consts.tile([P, N], BF16)
        nc.sync.dma_start(ptn2_sb[:], ptn2)
        at2_sb = consts.tile([P, I_LEN // 2], F32)
        nc.sync.dma_start(at2_sb[:], at2)
        w2pk_sb = consts.tile([P, 2 * H], BF16)
        nc.sync.dma_start(w2pk_sb[:], w2pk)
        ident = consts.tile([P, P], BF16)
        make_identity(nc, ident[:])

        kT_sb = consts.tile([P, 6, N], BF16)
        va_sb = consts.tile([P, 8, H * 65], BF16)
        qT_sb = consts.tile([P, 6, I_LEN], BF16)
        aT_sb = consts.tile([P, 6, I_LEN], BF16)

        # ones columns of va (col h*65+64 per token-chunk)
        nc.gpsimd.memset(
            va_sb[:].rearrange("p t (h e) -> p t h e", h=H, e=65)[:, :, :, 64:65],
            1.0,
        )

        # ---- h2 production (DVE): relu(A_col + ptn2), 8 pairs per tile ----
        h2t = []
        for t in range(16):
            ht = h2p.tile([P, 8, N], BF16, tag="h2")
            for s in range(8):
                pg = t * 8 + s
                nc.vector.tensor_scalar(
                    ht[:, s, :], ptn2_sb[:], at2_sb[:, pg:pg + 1], 0.0,
                    ALU.add, ALU.max,
                )
            h2t.append(ht)

        # ---- qkv projections ----
        # k^T [768 feat, 1024 tok]
        for fc in range(6):
            for jh in range(2):
                ps = mmp.tile([P, 512], F32, tag="mm")
                for cc in range(6):
                    nc.tensor.matmul(
                        ps[:],
                        kwT_sb[:, cc, fc * P:(fc + 1) * P],
                        xT_sb[:, cc, jh * 512:(jh + 1) * 512],
                        start=(cc == 0),
                        stop=(cc == 5),
                    )
                nc.gpsimd.tensor_copy(kT_sb[:, fc, jh * 512:(jh + 1) * 512], ps[:])
        # q^T for the core's i-slice (tokens 0:256 after rotation)
        for fc in range(6):
            ps = mmp.tile([P, I_LEN], F32, tag="mm")
            for cc in range(6):
                nc.tensor.matmul(
                    ps[:],
                    qwT_sb[:, cc, fc * P:(fc + 1) * P],
                    xT_sb[:, cc, 0:I_LEN],
                    start=(cc == 0),
                    stop=(cc == 5),
                )
            nc.scalar.copy(qT_sb[:, fc, :], ps[:])
        # v natural [tok, feat] into 65-column head slots
        for tci in range(8):
            for oh in range(2):
                ps = mmp.tile([P, 384], F32, tag="mm")
                for cc in range(6):
                    nc.tensor.matmul(
                        ps[:],
                        xT_sb[:, cc, tci * P:(tci + 1) * P],
                        vwT_sb[:, cc, oh * 384:(oh + 1) * 384],
                        start=(cc == 0),
                        stop=(cc == 5),
                    )
                dst = va_sb[:, tci, oh * 390:oh * 390 + 390].rearrange(
                    "p (h e) -> p h e", h=6, e=65
                )[:, :, 0:64]
                if oh == 0:
                    nc.gpsimd.tensor_copy(dst, ps[:])
                else:
                    nc.scalar.copy(dst, ps[:])

        # ---- attention (transposed) ----
        def avcol(h):
            return h * 65 if h < 7 else 512 + (h - 7) * 65

        for ic in range(2):
            for sg in range(2):
                expt = expp.tile([P, 8, 768], BF16, tag="exp")
                for jc in range(8):
                    SS = ssp.tile([P, 1024], F32, tag="ss")
                    # bias matmuls first (start=True covers each bank)
                    for s32 in range(2):
                        for ph in range(16):
                            pg = ic * 64 + sg * 32 + s32 * 16 + ph
                            ht = h2t[pg // 8]
                            nc.tensor.matmul(
                                SS[:, s32 * 512 + 24 * ph:s32 * 512 + 24 * ph + 24],
                                ht[:, pg % 8, jc * P:(jc + 1) * P],
                                w2pk_sb[:],
                                start=(ph == 0),
                                stop=False,
                            )
                    # scores accumulate on top: col = il*12 + h
                    for s32 in range(2):
                        ssb = SS[:, s32 * 512:s32 * 512 + 384].rearrange(
                            "p (i h) -> p i h", i=32, h=H
                        )
                        i0 = ic * 128 + sg * 64 + s32 * 32
                        for h in range(12):
                            off = (h % 2) * 64
                            fc = h // 2
                            nc.tensor.matmul(
                                ssb[:, :, h],
                                kT_sb[off:off + 64, fc, jc * P:(jc + 1) * P],
                                qT_sb[off:off + 64, fc, i0:i0 + 32],
                                start=False,
                                stop=(h == 11),
                            )
                    # exp -> expST staging: col = h*64 + s32*32 + il
                    src = SS[:].rearrange("p (b c) -> p b c", b=2, c=512)[:, :, 0:384]
                    dst = expt[:, jc, :].rearrange(
                        "p (h s i) -> p s i h", h=H, s=2, i=32
                    )
                    nc.scalar.activation(dst, src, ACTF.Exp, bias=0.0, scale=1.0)
                # AV for this 64-row i-block (rows 0:64 of avps)
                avps = avp.tile([64, 1024], F32, tag="av")
                for jc in range(8):
                    for h in range(12):
                        hc = avcol(h)
                        nc.tensor.matmul(
                            avps[:, hc:hc + 65],
                            expt[:, jc, h * 64:(h + 1) * 64],
                            va_sb[:, jc, h * 65:h * 65 + 65],
                            start=(jc == 0 and h in (0, 7)),
                            stop=(jc == 7 and h in (6, 11)),
                        )

                # normalize + pack O natural [64 i, 768] bf16
                rc = rcp.tile([64, H], F32, tag="rc")
                av_view = avps[:].rearrange("p (b c) -> p b c", b=2, c=512)
                nc.vector.reciprocal(
                    rc[:, 0:7],
                    av_view[:, 0, 0:455].rearrange(
                        "p (h e) -> p h e", h=7, e=65)[:, :, 64],
                )
                nc.vector.reciprocal(
                    rc[:, 7:12],
                    av_view[:, 1, 0:325].rearrange(
                        "p (h e) -> p h e", h=5, e=65)[:, :, 64],
                )
                an = anp.tile([64, C], BF16, tag="an")
                for h in range(12):
                    hc = avcol(h)
                    nc.gpsimd.tensor_scalar_mul(
                        an[:, h * 64:(h + 1) * 64],
                        avps[:, hc:hc + 64],
                        rc[:, h:h + 1],
                    )
                # transpose O chunk [64 i, 128 c] -> aT [128 c, 64 i]
                i0 = ic * 128 + sg * 64
                for cc in range(6):
                    pst = mmp.tile([P, 64], BF16, tag="mm")
                    nc.tensor.transpose(
                        pst[:], an[:, cc * P:(cc + 1) * P], ident[0:64, 0:64]
                    )
                    nc.vector.tensor_copy(aT_sb[:, cc, i0:i0 + 64], pst[:])
            # out projection
            for oh in range(2):
                ps = mmp.tile([P, 384], F32, tag="mm")
                for cc in range(6):
                    nc.tensor.matmul(
                        ps[:],
                        aT_sb[:, cc, ic * P:(ic + 1) * P],
                        pwT_sb[:, cc, oh * 384:(oh + 1) * 384],
                        start=(cc == 0),
                        stop=(cc == 5),
                    )
                oc = outp.tile([P, 384], F32, tag="oc")
                nc.scalar.copy(oc[:], ps[:])
                nc.gpsimd.dma_start(
                    out[ic * P:(ic + 1) * P, oh * 384:(oh + 1) * 384], oc[:]
                )

    nc.compile()
    return nc


def _prep_inputs(x, coords_3d, qkv_w, proj_w, mlp_w1, mlp_b1, mlp_w2):
    bf = ml_dtypes.bfloat16
    in_maps = []
    qw = (qkv_w[0:C] * (HD ** -0.5)).astype(np.float32)
    kw = qkv_w[C:2 * C]
    vw = qkv_w[2 * C:3 * C]
    qwT = np.ascontiguousarray(qw.T).astype(bf)
    kwT = np.ascontiguousarray(kw.T).astype(bf)
    vwT = np.ascontiguousarray(vw.T).astype(bf)
    pwT = np.ascontiguousarray(proj_w.T).astype(bf)
    # w2pk[par2*64+d, par*12+h] = (par==par2) * w2[h, d]
    w2pk = np.zeros((P, 2 * H), np.float32)
    w2pk[0:HID, 0:H] = mlp_w2.T
    w2pk[HID:2 * HID, H:2 * H] = mlp_w2.T
    w2pk = w2pk.astype(bf)

    for b in range(B):
        cb = coords_3d[b].astype(np.float32)
        mv = cb.max(axis=0) - cb.min(axis=0) + 1e-6
        cn = cb / mv
        Pm = cn @ mlp_w1.T.astype(np.float32)          # (1024, 64)
        Am = Pm + mlp_b1.astype(np.float32)            # (1024, 64)
        nPmT = -Pm.T                                   # (64, 1024)
        xT_b = np.ascontiguousarray(x[b].T).astype(np.float32)  # (768, 1024)
        for s in range(NSLICE):
            i0 = s * I_LEN
            # token rotation: column j' holds token (j' + i0) % N
            xTr = np.roll(xT_b, -i0, axis=1).astype(bf)
            ptn2 = np.empty((P, N), np.float32)
            ptn2[0:HID] = np.roll(nPmT, -i0, axis=1)
            ptn2[HID:2 * HID] = ptn2[0:HID]
            at2 = np.empty((P, I_LEN // 2), np.float32)
            Al = Am[i0:i0 + I_LEN]
            at2[0:HID] = Al[0::2].T
            at2[HID:2 * HID] = Al[1::2].T
            in_maps.append({
                "xT": xTr,
                "qwT": qwT,
                "kwT": kwT,
                "vwT": vwT,
                "pwT": pwT,
                "ptn2": ptn2.astype(bf),
                "at2": at2.astype(np.float32),
                "w2pk": w2pk,
            })
    return in_maps


def kernel(x, coords_3d, qkv_w, proj_w, proj_b, mlp_w1, mlp_b1, mlp_w2, mlp_b2):
    global LAST_EXEC_NS, LAST_RESULTS
    x = np.asarray(x, np.float32)
    coords_3d = np.asarray(coords_3d, np.float32)
    qkv_w = np.asarray(qkv_w, np.float32)
    proj_w = np.asarray(proj_w, np.float32)
    proj_b = np.asarray(proj_b, np.float32)
    mlp_w1 = np.asarray(mlp_w1, np.float32)
    mlp_b1 = np.asarray(mlp_b1, np.float32)
    mlp_w2 = np.asarray(mlp_w2, np.float32)

    if "nc" not in _CACHE:
        _CACHE["nc"] = _build_program()
    nc = _CACHE["nc"]

    in_maps = _prep_inputs(x, coords_3d, qkv_w, proj_w, mlp_w1, mlp_b1, mlp_w2)
    trace = bool(int(os.environ.get("KERNEL_TRACE", "0")))
    res = bass_utils.run_bass_kernel_spmd(
        nc, in_maps, list(range(8)), trace=trace
    )
    LAST_EXEC_NS = res.exec_time_ns
    LAST_RESULTS = res
    full = np.empty((B, N, C), np.float32)
    ci = 0
    for b in range(B):
        for s in range(NSLICE):
            full[b, s * I_LEN:(s + 1) * I_LEN] = res.results[ci]["out"]
            ci += 1
    full += proj_b[None, None, :]
    return full
